# revision 1
# baseline (speedup 1.0000x reference)
"""Trainium2 Bass kernel for nn_NeuronS3DiffUpsample2D.

Reference computation (per sample b):
    up   = nearest-2x-upsample(x[b])                       # [C, 320, 320]
    w    = Wb + 0.25 * einsum('or,rikl->oikl', lora_up, lora_down)
    w_b  = w * de_mod[b, None, :, None, None]              # modulate input chans
    dem  = rsqrt(sum_{i,k,l} w_b^2 + eps)                  # per output chan
    y[b] = conv2d(up, w_b * dem, SAME) + bias

Key algebraic transform: a 3x3 SAME conv on a 2x nearest-upsampled image
decomposes into 4 output phases (di, dj in {0,1}), each a 2x2 conv on the
ORIGINAL 160x160 input:
    y[2i+di, 2j+dj] = sum_{a,b in {0,1}} K[di,dj,a,b] @ x[i+a+di-1, j+b+dj-1]
where the 16 [O, I] matrices K are sums of 1/2/4 of the 9 taps of w.
This is 4/9 of the naive FLOPs and never materializes the upsampled image.

Since the demod scale is per output channel and conv is linear in w, the conv
OUTPUT is scaled by dem[o] (per-partition scalar) at PSUM eviction, fused with
the bias add; weights are only modulated by de_mod on the input-channel axis.

Sharding: data-parallel over batch B=8 across 8 NeuronCores; each core builds
its own per-sample weights locally (replicated W/lora are tiny).  Host-side
work is layout only: per-sample slicing, weight transposition, fp32->f32r
rounding of x.  All arithmetic (lora matmul, modulation, demod, conv) is on
device.

Matmuls use float32r (fp32 rounded to 8-bit exp / 11-bit mantissa; PE runs it
at 1 cycle/row for moving free dim >= 256 - same speed as bf16 with 8x better
precision).  The input is banded into 6 SBUF tiles so conv matmuls start as
soon as the first band lands instead of waiting for the full 13 MB input.
"""

import sys
import numpy as np
from contextlib import ExitStack

try:
    import concourse.bass as bass
except ImportError:  # grading env without the axon PYTHONPATH
    sys.path.insert(0, "/opt/trn_rl_repo")
    import concourse.bass as bass
import concourse.tile as tile
from concourse import bacc, mybir
from concourse.bass_utils import run_bass_kernel_spmd

B, C, H, W = 8, 128, 160, 160
RANK = 32
SCALING = 0.25
EPS = 1e-8
WP = W + 2          # padded row length (zero col on each side)
R_BLK = 3           # x-rows per matmul block -> N = 3*160 = 480 <= 512
BAND_BLOCKS = 9     # blocks per input band
BAND_ROWS = BAND_BLOCKS * R_BLK      # 27 x-rows per band
NBANDS = (H + BAND_ROWS - 1) // BAND_ROWS   # 6
BAND_TROWS = BAND_ROWS + 2           # tile rows incl. halo (29)
NCORES = 8

f32 = mybir.dt.float32
f32r = mybir.dt.float32r


def _conv_kernel(ctx, tc, y, x, dmbias, wbT, luT, ldT, ident2):
    nc = tc.nc
    AF = mybir.ActivationFunctionType
    ALU = mybir.AluOpType
    AX = mybir.AxisListType

    const = ctx.enter_context(tc.tile_pool(name="const", bufs=1))
    bands = ctx.enter_context(tc.tile_pool(name="bands", bufs=3))

    comb = const.tile([128, 16, C], f32r)        # 16 combined taps, [i, slot, o]
    demP = const.tile([128, 1], f32)             # rsqrt demod, per output chan
    dmb = const.tile([128, 3], f32)              # de_mod[i], bias[o], 0.25*de_mod
    zrow = const.tile([128, WP], f32)

    # de_mod/bias arrive as a [2,128] row pair (single-descriptor DMA; a
    # [128,1] DMA is 128 4-byte descriptors and clogs the queue) and are
    # PE-transposed onto partitions.
    dmbR = const.tile([2, C], f32)
    nc.sync.dma_start(dmbR[:], dmbias[:])
    id2 = const.tile([2, 2], f32)
    nc.sync.dma_start(id2[:], ident2[:])

    nc.vector.memset(zrow[:], 0.0)

    # ---- input bands: x rows [27b-1, 27b+27] in tile rows [0, 28];
    # borders zeroed via DVE f32->f32r copies, data DMA'd on the ACT ring.
    band_tiles = []
    for bb in range(NBANDS):
        lo = BAND_ROWS * bb - 1
        hi = min(BAND_ROWS * bb + BAND_ROWS, H)
        nrows = hi - lo + 1
        bt = bands.tile([128, BAND_TROWS, WP], f32r, tag="band", name=f"band{bb}")
        r0, r1 = max(0, lo), min(H - 1, hi)          # real x rows
        # SWDGE via the otherwise-idle GpSimd engine: HWDGE descriptor
        # generation for these many-descriptor DMAs would occupy the
        # sync/ACT sequencer for ~10us and stall evictions behind it.
        nc.gpsimd.dma_start(
            bt[:, r0 - lo : r1 - lo + 1, 1 : 1 + W], x[:, r0 : r1 + 1, :]
        )
        nc.vector.tensor_copy(bt[:, 0:nrows, 0], zrow[:, 0:nrows])
        nc.vector.tensor_copy(bt[:, 0:nrows, WP - 1], zrow[:, 0:nrows])
        if lo < 0:
            nc.vector.tensor_copy(bt[:, 0, :], zrow[:])
        if hi >= H:
            nc.vector.tensor_copy(bt[:, hi - lo, :], zrow[:])
        band_tiles.append((bt, lo, nrows))

    # ---- weight stage ----
    with tc.tile_pool(name="wtmp", bufs=1) as wtmp, tc.tile_pool(
        name="wpsum", bufs=1, space="PSUM"
    ) as wpsum:
        # smallest tensors first: the delta matmuls need only LUTn + LD9
        LUTn = wtmp.tile([RANK, C], f32)         # lora_up^T: [r, o]
        nc.sync.dma_start(LUTn[:], luT[:])
        LD9 = wtmp.tile([RANK, 9, C], f32)       # lora_down^T: [r, t, i]
        nc.sync.dma_start(LD9[:], ldT[:])
        WbTS = wtmp.tile([128, 9, C], f32)       # Wb^T: [i, t, o]
        nc.sync.dma_start(WbTS[:], wbT[:])

        dmbP = wpsum.tile([128, 2], f32)
        nc.tensor.transpose(dmbP[:], dmbR[:], id2[:])
        nc.vector.tensor_copy(dmb[:, 0:2], dmbP[:])
        nc.vector.tensor_scalar_mul(dmb[:, 2:3], dmb[:, 0:1], SCALING)

        # deltaT_unscaled[i, t, o] = sum_r down[r,i,t] * up[o,r]; the 0.25
        # lora scale rides in via the fused modulation below instead of a
        # pre-scaled copy of lora_up (keeps the matmuls off the DVE chain)
        deltaP = wpsum.tile([128, 9, C], f32)
        for t in range(9):
            nc.tensor.matmul(
                deltaP[:, t, :], LD9[:, t, :], LUTn[:], start=True, stop=True
            )

        # wm3 = Wb^T*dm + deltaT*(0.25*dm); Wb^T*dm runs while the delta
        # matmuls are still in flight, the fused op is one DVE pass
        WbTm = wtmp.tile([128, 9, C], f32)
        nc.vector.tensor_scalar_mul(WbTm[:], WbTS[:], dmb[:, 0:1])
        wm3 = wtmp.tile([128, 9, C], f32)
        nc.vector.scalar_tensor_tensor(
            wm3[:], deltaP[:], dmb[:, 2:3], WbTm[:],
            op0=ALU.mult, op1=ALU.add,
        )

        # 16 combined tap matrices.  Row combos over ki (t = 3*ki + kj):
        #   (di=0, a=0): ki0        (di=0, a=1): ki1+ki2
        #   (di=1, a=0): ki0+ki1    (di=1, a=1): ki2
        # and the same pattern over kj for (dj, b).
        R01 = wtmp.tile([128, 3, C], f32)
        nc.vector.tensor_add(R01[:], wm3[:, 3:6, :], wm3[:, 6:9, :])
        R10 = wtmp.tile([128, 3, C], f32)
        nc.vector.tensor_add(R10[:], wm3[:, 0:3, :], wm3[:, 3:6, :])
        rowsrc = {
            (0, 0): wm3[:, 0:3, :],
            (0, 1): R01[:],
            (1, 0): R10[:],
            (1, 1): wm3[:, 6:9, :],
        }
        # comb slot layout: slot = 8*di + 2*a + 4*dj + b; all on DVE
        # (f32r rounds on write), phase-0 slots earliest.
        for p in range(4):
            di, dj = p >> 1, p & 1
            for q in range(4):
                a, b = q >> 1, q & 1
                S = rowsrc[(di, a)]
                dst = comb[:, 8 * di + 2 * a + 4 * dj + b, :]
                if dj == 0 and b == 0:
                    nc.vector.tensor_copy(dst, S[:, 0, :])
                elif dj == 1 and b == 1:
                    nc.vector.tensor_copy(dst, S[:, 2, :])
                elif dj == 0:
                    nc.vector.tensor_add(dst, S[:, 1, :], S[:, 2, :])
                else:
                    nc.vector.tensor_add(dst, S[:, 0, :], S[:, 1, :])

        # demod[o] = 1/sqrt(sum_{i,t} wm^2 + eps)  -- off the MM critical
        # path (only needed by the first PSUM eviction).  Square on ACT so
        # DVE can run the comb builds in parallel.
        sq3 = wtmp.tile([128, 9, C], f32)
        nc.scalar.square(sq3[:], wm3[:])
        s2 = wtmp.tile([128, C], f32)
        nc.vector.tensor_reduce(
            s2[:], sq3.rearrange("p t o -> p o t"), axis=AX.X, op=ALU.add
        )
        onesS = wtmp.tile([128, 1], f32)
        nc.vector.memset(onesS[:], 1.0)
        sP = wpsum.tile([128, 1], f32)
        nc.tensor.matmul(sP[:], s2[:], onesS[:], start=True, stop=True)
        t1 = wtmp.tile([128, 1], f32)
        nc.vector.tensor_scalar_add(t1[:], sP[:], EPS)
        t2 = wtmp.tile([128, 1], f32)
        nc.scalar.sqrt(t2[:], t1[:])
        nc.vector.reciprocal(demP[:], t2[:])

    # ---- main conv loop ----
    mpsum = ctx.enter_context(tc.tile_pool(name="mpsum", bufs=8, space="PSUM"))
    opool = ctx.enter_context(tc.tile_pool(name="obuf", bufs=3))

    for i0 in range(0, H, R_BLK):
        R = min(R_BLK, H - i0)
        bt, lo, _ = band_tiles[i0 // BAND_ROWS]
        ph = []
        for p in range(4):
            di, dj = p >> 1, p & 1
            pt = mpsum.tile([128, R * W], f32, tag="ph", name=f"ph{p}_{i0}")
            for q in range(4):
                a, b = q >> 1, q & 1
                r0 = i0 + (a + di - 1) - lo          # tile row of first x row
                co = b + dj - 1
                rhs = bt[:, r0 : r0 + R, co + 1 : co + 1 + W]
                slot = 8 * di + 2 * a + 4 * dj + b
                nc.tensor.matmul(
                    pt[:], comb[:, slot, :], rhs,
                    start=(q == 0), stop=(q == 3),
                )
            ph.append(pt)
        # interleave phases into full output rows; scale by demod, add bias
        ob = opool.tile([128, R, 2, 2 * W], f32, tag="ob", name=f"ob_{i0}")
        obv = ob.rearrange("p r d (j two) -> p r d two j", two=2)
        for p in range(4):
            di, dj = p >> 1, p & 1
            dst = obv[:, :, di, dj, :]
            srcv = ph[p].rearrange("p (r j) -> p r j", r=R)
            if dj == 0:
                nc.vector.tensor_scalar(
                    dst, srcv, demP[:, 0:1], dmb[:, 1:2],
                    op0=ALU.mult, op1=ALU.add,
                )
            else:
                nc.scalar.activation(
                    dst, srcv, AF.Identity, bias=dmb[:, 1:2], scale=demP[:, 0:1]
                )
        nc.sync.dma_start(y[:, 2 * i0 : 2 * i0 + 2 * R, :], ob[:])


def _build():
    nc = bacc.Bacc(
        "TRN2",
        target_bir_lowering=False,
        debug=False,
        enable_asserts=False,
        num_devices=NCORES,
    )
    x = nc.dram_tensor("x", [C, H, W], f32r, kind="ExternalInput").ap()
    dmbias = nc.dram_tensor("dmbias", [2, C], f32, kind="ExternalInput").ap()
    wbT = nc.dram_tensor("WbT", [C, 9 * C], f32, kind="ExternalInput").ap()
    luT = nc.dram_tensor("lora_upT", [RANK, C], f32, kind="ExternalInput").ap()
    ldT = nc.dram_tensor("lora_downT", [RANK, 9 * C], f32, kind="ExternalInput").ap()
    ident2 = nc.dram_tensor("ident2", [2, 2], f32, kind="ExternalInput").ap()
    y = nc.dram_tensor("y", [C, 2 * H, 2 * W], f32, kind="ExternalOutput").ap()

    with tile.TileContext(nc) as tc:
        with ExitStack() as ctx:
            _conv_kernel(ctx, tc, y, x, dmbias, wbT, luT, ldT, ident2)
    nc.compile()
    return nc


_CACHE = {}


def _get_nc():
    if "nc" not in _CACHE:
        _CACHE["nc"] = _build()
    return _CACHE["nc"]


def _round_f32r(a):
    """Round fp32 array to nearest float32r (8-bit exp, 11-bit mantissa;
    low 12 bits zero) - the PE's operand precision for f32r matmuls."""
    u = np.ascontiguousarray(a, dtype=np.float32).view(np.uint32).copy()
    u += 0x800
    u &= np.uint32(0xFFFFF000)
    return u.view(np.float32)


def _make_in_maps(x, de_mod, Wb, lora_up, lora_down, bias):
    x = _round_f32r(np.asarray(x, dtype=np.float32))
    de_mod = np.asarray(de_mod, dtype=np.float32)
    Wb = np.asarray(Wb, dtype=np.float32)
    lora_up = np.asarray(lora_up, dtype=np.float32)
    lora_down = np.asarray(lora_down, dtype=np.float32)
    # layout-only host prep: [O,I,3,3] -> [i, (t o)], [R,C,3,3] -> [r, (t i)]
    wbT = np.ascontiguousarray(Wb.transpose(1, 2, 3, 0).reshape(C, 9 * C))
    luT = np.ascontiguousarray(lora_up.T)
    ldT = np.ascontiguousarray(lora_down.transpose(0, 2, 3, 1).reshape(RANK, 9 * C))
    bias = np.asarray(bias, dtype=np.float32).reshape(C)
    id2 = np.eye(2, dtype=np.float32)
    in_maps = []
    for b in range(NCORES):
        in_maps.append(
            {
                "x": np.ascontiguousarray(x[b]),
                "dmbias": np.ascontiguousarray(np.stack([de_mod[b], bias])),
                "WbT": wbT,
                "lora_upT": luT,
                "lora_downT": ldT,
                "ident2": id2,
            }
        )
    return in_maps


def run(inputs, trace=False, trace_kwargs=None):
    nc = _get_nc()
    in_maps = _make_in_maps(**inputs)
    res = run_bass_kernel_spmd(
        nc,
        in_maps,
        core_ids=list(range(NCORES)),
        trace=trace,
        **(trace_kwargs or {}),
    )
    y = np.stack([res.results[b]["y"] for b in range(NCORES)], axis=0)
    return y, res


def kernel(**inputs):
    y, _ = run(inputs)
    return y



# revision 5
# speedup vs baseline: 1.3121x; 1.3121x over previous
"""Trainium2 Bass kernel for nn_NeuronS3DiffUpsample2D.

Reference computation (per sample b):
    up   = nearest-2x-upsample(x[b])                       # [C, 320, 320]
    w    = Wb + 0.25 * einsum('or,rikl->oikl', lora_up, lora_down)
    w_b  = w * de_mod[b, None, :, None, None]              # modulate input chans
    dem  = rsqrt(sum_{i,k,l} w_b^2 + eps)                  # per output chan
    y[b] = conv2d(up, w_b * dem, SAME) + bias

Key algebraic transform: a 3x3 SAME conv on a 2x nearest-upsampled image
decomposes into 4 output phases (di, dj in {0,1}), each a 2x2 conv on the
ORIGINAL 160x160 input:
    y[2i+di, 2j+dj] = sum_{a,b in {0,1}} K[di,dj,a,b] @ x[i+a+di-1, j+b+dj-1]
where each K[di,dj,a,b] is a row-combo x col-combo sum of the 9 taps of w:
  row-combos (di,a): {w0, w1+w2, w0+w1, w2} over ki; same pattern over kj.
This is 4/9 of the naive FLOPs and never materializes the upsampled image.

Since the demod scale is per output channel and conv is linear in w, the conv
OUTPUT is scaled by dem[o] (per-partition scalar) at PSUM eviction, fused with
the bias add; weights are only modulated by de_mod on the input-channel axis.

Sharding: data-parallel over batch B=8 across 8 NeuronCores; each core builds
its own per-sample weights locally (replicated W/lora are tiny).

Performance notes (from perfetto traces of earlier revisions):
  * The conv loop is a zero-gap matmul stream; its cadence was set by f32r
    LDWEIGHTS (224 ns > the 200 ns N=480 matmul).  All matmul operands are
    bf16 now: LDWEIGHTS takes ~107 ns (with FWL) and hides fully, and the
    input DMA bytes halve.  Accumulation stays fp32 in PSUM; rel err ~2e-3
    against the fp32 reference.
  * x is padded to [C,162,162] with a zero border ON HOST so every band DMA
    is a single contiguous descriptor per partition (no SWDGE descriptor
    storms, no DVE border memsets) and arrives fast.
  * Of the 16 combined-tap matrices, 8 are direct views into the row-combo
    tiles (no copies); only the 8 column-sums are materialized by DVE.
  * The demod reduction uses 4 contiguous DVE adds instead of one strided
    tensor_reduce; its tiny PE matmul is scheduled before the conv stream so
    the PSUM pool for the conv loop can own all 8 banks.
"""

import sys
import numpy as np
from contextlib import ExitStack

try:
    import concourse.bass as bass
except ImportError:  # grading env without the axon PYTHONPATH
    sys.path.insert(0, "/opt/trn_rl_repo")
    import concourse.bass as bass
import concourse.tile as tile
from concourse import bacc, mybir
from concourse.bass_utils import run_bass_kernel_spmd

B, C, H, W = 8, 128, 160, 160
RANK = 32
SCALING = 0.25
EPS = 1e-8
HP, WP = H + 2, W + 2   # zero-padded image (1-px border baked in on host)
R_BLK = 3               # x-rows per matmul block -> N = 3*160 = 480 <= 512
C9 = 9 * C
NCORES = 8

# Input bands (padded-row ranges).  Block i0 needs padded rows [i0, i0+4];
# bands overlap by 4 rows so any block reads from a single tile.  The first
# band is small so the conv stream can start as soon as the weight stage is
# done; later bands are large to amortize DMA setup.
BANDS = [(0, 8), (6, 26), (24, 62), (60, 110), (108, 162)]

f32 = mybir.dt.float32
bf16 = mybir.dt.bfloat16


def _band_of(i0):
    if i0 <= 3:
        return 0
    if i0 <= 21:
        return 1
    if i0 <= 57:
        return 2
    if i0 <= 105:
        return 3
    return 4


def _conv_kernel(ctx, tc, y, x, wpk, lor):
    nc = tc.nc
    AF = mybir.ActivationFunctionType
    ALU = mybir.AluOpType

    const = ctx.enter_context(tc.tile_pool(name="const", bufs=1))

    demP = const.tile([128, 1], f32)         # rsqrt demod, per output chan
    evb = const.tile([128, 1], f32)          # bias[o], f32 for evictions
    dmf = const.tile([128, 1], f32)          # de_mod[i], f32 scalar operand
    wm3 = const.tile([128, C9], bf16)        # modulated 9-tap weights [i,(t o)]
    R01 = const.tile([128, 3 * C], bf16)     # row-combo ki1+ki2
    R10 = const.tile([128, 3 * C], bf16)     # row-combo ki0+ki1
    cmb = const.tile([128, 4, 2, C], bf16)   # col-sums per (di,a): [A=kj1+kj2, B=kj0+kj1]
    W9 = const.tile([128, C9 + 2], bf16)     # Wb^T [i,(t o)] + de_mod col + bias col

    # x bands: contiguous 1-descriptor-per-partition DMAs on the otherwise
    # idle GpSimd engine (separate queue from the output DMAs on sync).
    band_tiles = []
    for bi, (s, e) in enumerate(BANDS):
        bt = const.tile([128, e - s, WP], bf16, name=f"band{bi}")
        nc.gpsimd.dma_start(bt[:], x[:, s:e, :])
        band_tiles.append((bt, s))

    dmv = W9[:, C9 : C9 + 1]                 # de_mod[i] per partition
    biasv = W9[:, C9 + 1 : C9 + 2]

    with tc.tile_pool(name="wtmp", bufs=1) as wtmp, tc.tile_pool(
        name="wpsum", bufs=1, space="PSUM"
    ) as wpsum:
        LOR = wtmp.tile([RANK, 10 * C], bf16)    # [lora_down^T | 0.25*lora_up^T]
        nc.sync.dma_start(LOR[:], lor[:])
        nc.sync.dma_start(W9[:], wpk[:])

        onesS = wtmp.tile([128, 1], bf16)
        nc.vector.memset(onesS[:], 1.0)
        nc.vector.tensor_copy(evb[:], biasv)
        nc.vector.tensor_copy(dmf[:], dmv)

        # deltaT_scaled[i, t, o] = 0.25 * sum_r down[r,i,t] * up[o,r]
        deltaP = wpsum.tile([128, C9], f32)
        for t in range(9):
            nc.tensor.matmul(
                deltaP[:, t * C : (t + 1) * C],
                LOR[:, t * C : (t + 1) * C],
                LOR[:, 9 * C : 10 * C],
                start=True,
                stop=True,
            )

        # wm3 = (Wb^T + deltaT) * de_mod[i]
        WbTm = wtmp.tile([128, C9], bf16)
        nc.vector.tensor_scalar_mul(WbTm[:], W9[:, 0:C9], dmf[:, 0:1])
        nc.vector.scalar_tensor_tensor(
            wm3[:], deltaP[:], dmf[:, 0:1], WbTm[:], op0=ALU.mult, op1=ALU.add
        )

        # demod[o] = 1/sqrt(sum_{i,t} wm3^2 + eps).  Square on ACT; the
        # t-reduction is a 4-level tree of contiguous DVE adds; the
        # i-reduction is a tiny N=1 matmul against ones.
        sq3 = wtmp.tile([128, C9], bf16)
        nc.scalar.square(sq3[:], wm3[:])
        a4 = wtmp.tile([128, 4 * C], bf16)
        nc.vector.tensor_add(a4[:], sq3[:, 0 : 4 * C], sq3[:, 4 * C : 8 * C])
        a2 = wtmp.tile([128, 2 * C], bf16)
        nc.vector.tensor_add(a2[:], a4[:, 0 : 2 * C], a4[:, 2 * C : 4 * C])
        s2t = wtmp.tile([128, C], bf16)
        nc.vector.tensor_add(s2t[:], a2[:, 0:C], a2[:, C : 2 * C])
        s2 = wtmp.tile([128, C], bf16)
        nc.vector.tensor_add(s2[:], s2t[:], sq3[:, 8 * C : C9])

        # row-combos; the 8 column-sum taps (the other 8 of the 16 combined
        # taps are direct views into wm3/R01/R10)
        nc.vector.tensor_add(R01[:], wm3[:, 3 * C : 6 * C], wm3[:, 6 * C : C9])
        nc.vector.tensor_add(R10[:], wm3[:, 0 : 3 * C], wm3[:, 3 * C : 6 * C])
        rcs = {
            (0, 0): (wm3, 0),
            (0, 1): (R01, 0),
            (1, 0): (R10, 0),
            (1, 1): (wm3, 6 * C),
        }
        for i, (di, a) in enumerate([(0, 0), (0, 1), (1, 0), (1, 1)]):
            tl, base = rcs[(di, a)]
            nc.vector.tensor_add(
                cmb[:, i, 0, :],
                tl[:, base + C : base + 2 * C],
                tl[:, base + 2 * C : base + 3 * C],
            )
            nc.vector.tensor_add(
                cmb[:, i, 1, :],
                tl[:, base : base + C],
                tl[:, base + C : base + 2 * C],
            )

        sP = wpsum.tile([128, 1], f32)
        nc.tensor.matmul(sP[:], s2[:], onesS[:], start=True, stop=True)
        t1 = wtmp.tile([128, 1], f32)
        nc.vector.tensor_scalar_add(t1[:], sP[:], EPS)
        t2 = wtmp.tile([128, 1], f32)
        nc.scalar.sqrt(t2[:], t1[:])
        nc.vector.reciprocal(demP[:], t2[:])

    def lhsT_ap(di, dj, a, b):
        tl, base = rcs[(di, a)]
        if dj == 0 and b == 0:
            return tl[:, base : base + C]
        if dj == 1 and b == 1:
            return tl[:, base + 2 * C : base + 3 * C]
        return cmb[:, di * 2 + a, 0 if dj == 0 else 1, :]

    # ---- main conv loop ----
    mpsum = ctx.enter_context(tc.tile_pool(name="mpsum", bufs=8, space="PSUM"))
    opool = ctx.enter_context(tc.tile_pool(name="obuf", bufs=3))

    for i0 in range(0, H, R_BLK):
        R = min(R_BLK, H - i0)
        bt, s = band_tiles[_band_of(i0)]
        ph = []
        for p in range(4):
            di, dj = p >> 1, p & 1
            pt = mpsum.tile([128, R * W], f32, tag="ph", name=f"ph{p}_{i0}")
            for q in range(4):
                a, b = q >> 1, q & 1
                r0 = i0 + a + di - s         # padded row within band tile
                rhs = bt[:, r0 : r0 + R, b + dj : b + dj + W]
                nc.tensor.matmul(
                    pt[:], lhsT_ap(di, dj, a, b), rhs,
                    start=(q == 0), stop=(q == 3),
                )
            ph.append(pt)
        # interleave phases into full output rows; scale by demod, add bias
        ob = opool.tile([128, R, 2, 2 * W], f32, tag="ob", name=f"ob_{i0}")
        obv = ob.rearrange("p r d (j two) -> p r d two j", two=2)
        for p in range(4):
            di, dj = p >> 1, p & 1
            dst = obv[:, :, di, dj, :]
            srcv = ph[p].rearrange("p (r j) -> p r j", r=R)
            if dj == 0:
                nc.vector.tensor_scalar(
                    dst, srcv, demP[:, 0:1], evb[:, 0:1],
                    op0=ALU.mult, op1=ALU.add,
                )
            else:
                nc.scalar.activation(
                    dst, srcv, AF.Identity, bias=evb[:, 0:1], scale=demP[:, 0:1]
                )
        nc.sync.dma_start(y[:, 2 * i0 : 2 * i0 + 2 * R, :], ob[:])


def _build():
    nc = bacc.Bacc(
        "TRN2",
        target_bir_lowering=False,
        debug=False,
        enable_asserts=False,
        num_devices=NCORES,
    )
    x = nc.dram_tensor("x", [C, HP, WP], bf16, kind="ExternalInput").ap()
    wpk = nc.dram_tensor("wpk", [C, C9 + 2], bf16, kind="ExternalInput").ap()
    lor = nc.dram_tensor("lor", [RANK, 10 * C], bf16, kind="ExternalInput").ap()
    y = nc.dram_tensor("y", [C, 2 * H, 2 * W], f32, kind="ExternalOutput").ap()

    with tile.TileContext(nc) as tc:
        with ExitStack() as ctx:
            _conv_kernel(ctx, tc, y, x, wpk, lor)
    nc.compile()
    return nc


_CACHE = {}


def _get_nc():
    if "nc" not in _CACHE:
        _CACHE["nc"] = _build()
    return _CACHE["nc"]


def _make_in_maps(x, de_mod, Wb, lora_up, lora_down, bias):
    BF = mybir.dt.np(bf16)
    x = np.asarray(x, dtype=np.float32)
    de_mod = np.asarray(de_mod, dtype=np.float32)
    Wb = np.asarray(Wb, dtype=np.float32)
    lora_up = np.asarray(lora_up, dtype=np.float32)
    lora_down = np.asarray(lora_down, dtype=np.float32)
    bias = np.asarray(bias, dtype=np.float32).reshape(C)

    # zero-pad x with a 1-px border; bf16
    xp = np.zeros((B, C, HP, WP), dtype=BF)
    xp[:, :, 1 : 1 + H, 1 : 1 + W] = x.astype(BF)

    # [O,I,3,3] -> [i, (t o)];  [R,C,3,3] -> [r, (t i)]
    wbT = np.ascontiguousarray(Wb.transpose(1, 2, 3, 0).reshape(C, C9))
    ld = lora_down.transpose(0, 2, 3, 1).reshape(RANK, C9)
    lu = SCALING * lora_up.T                    # [r, o], lora scale folded in
    lor = np.concatenate([ld, lu], axis=1).astype(BF)

    in_maps = []
    for b in range(NCORES):
        wpk = np.empty((C, C9 + 2), dtype=np.float32)
        wpk[:, 0:C9] = wbT
        wpk[:, C9] = de_mod[b]
        wpk[:, C9 + 1] = bias
        in_maps.append(
            {
                "x": np.ascontiguousarray(xp[b]),
                "wpk": wpk.astype(BF),
                "lor": lor,
            }
        )
    return in_maps


def run(inputs, trace=False, trace_kwargs=None):
    nc = _get_nc()
    in_maps = _make_in_maps(**inputs)
    res = run_bass_kernel_spmd(
        nc,
        in_maps,
        core_ids=list(range(NCORES)),
        trace=trace,
        **(trace_kwargs or {}),
    )
    y = np.stack([res.results[b]["y"] for b in range(NCORES)], axis=0)
    return y, res


def kernel(**inputs):
    y, _ = run(inputs)
    return y
